# revision 2
# baseline (speedup 1.0000x reference)
"""Transformer block (pre-LN causal MHA + GELU MLP) on 8 trn2 NeuronCores.

Sharding: core r handles batch b=r//4, group position p=r%4, owning q-tiles
{p, p+4, p+8, p+12} of sixteen 128-token tiles. Local slot u=0..3 holds
global tile G(u) = p + 4*(3-u) (DESCENDING), so for key-group m (keys
512m:512m+512) the consumer q-slots form a PREFIX of the token columns:
u <= 3-m. Score and AV matmuls then always start at column 0 of their PSUM
bank — real TRN2 requires matmul PSUM outputs to start 2KB bank-aligned.

Per-core key-tile work drops from 48 to 40 blocks of 128x128 and every
(head, query) finishes its softmax in one PSUM pass (no cross-phase stash).
K^T and V are exchanged via four per-slot AllGathers inside each 4-core
batch group, fired as soon as that slot's K/V are computed (earliest keys
first), so gathers hide under QKV compute.

Attention computes transposed scores S^T[k, q] = K.Q^T so the softmax
row-sum falls out of a ones-augmented V matmul; causal masks are
multiplicative 0/1 indicators built from a per-core qbase input and only
touch the last 128 columns (the diagonal slot) of each key-group.

Everything is bf16 with fp32 PSUM accumulation; LN gamma/beta and the
1/sqrt(D) score scale are folded into weights on the host.

Self-contained: hardcodes B=2, T=2048, C=1024, H=16, D=64, hidden=4096.
"""
import sys

if "/opt/trn_rl_repo" not in sys.path:
    sys.path.insert(0, "/opt/trn_rl_repo")

import numpy as np
import ml_dtypes

B, T, C, H = 2, 2048, 1024, 16
D = C // H            # 64
MH = 4 * C            # 4096 mlp hidden
EPS = 1e-5
P = 128
TOK = 512             # tokens per core
QT = 128              # tokens per q-tile (slot)
NS = 4                # slots per core
N_CORES = 8
SCALE = 1.0 / np.sqrt(D)

_CACHE: dict = {}


def _build(mock_cc=False, sim_gelu_relu=False, bisect=""):
    import concourse.tile as tile
    from concourse import bacc, mybir
    from concourse.masks import make_identity
    from contextlib import ExitStack

    F32 = mybir.dt.float32
    BF16 = mybir.dt.bfloat16
    I32 = mybir.dt.int32
    AF = mybir.ActivationFunctionType
    ALU = mybir.AluOpType

    nc = bacc.Bacc()

    # ---------------- I/O ----------------
    x_in = nc.declare_dram_parameter("x", [TOK, C], F32, isOutput=False)
    qbase_in = nc.declare_dram_parameter("qbase", [1, NS], F32, isOutput=False)
    w_attn = nc.declare_dram_parameter("w_attn", [C, 3 * C], BF16, isOutput=False)
    b_attn = nc.declare_dram_parameter("b_attn", [3 * C], F32, isOutput=False)
    w_o = nc.declare_dram_parameter("w_o", [C, C], BF16, isOutput=False)
    b_o = nc.declare_dram_parameter("b_o", [C], F32, isOutput=False)
    w_fc = nc.declare_dram_parameter("w_fc", [C, MH], BF16, isOutput=False)
    b_fc = nc.declare_dram_parameter("b_fc", [MH], F32, isOutput=False)
    w_fc2 = nc.declare_dram_parameter("w_fc2", [MH, C], BF16, isOutput=False)
    b_fc2 = nc.declare_dram_parameter("b_fc2", [C], F32, isOutput=False)
    out_ext = nc.declare_dram_parameter("out", [TOK, C], F32, isOutput=True)

    # internal DRAM for the per-slot collectives (indexed by local slot u;
    # slot u carries global key tiles 4*(3-u)..4*(3-u)+3)
    kt_in = [nc.dram_tensor(f"kt_in_{u}", [C, QT], BF16) for u in range(NS)]
    v_in = [nc.dram_tensor(f"v_in_{u}", [QT, C], BF16) for u in range(NS)]
    kt_all = [nc.dram_tensor(f"kt_all_{u}", [4 * C, QT], BF16) for u in range(NS)]
    v_all = [nc.dram_tensor(f"v_all_{u}", [4 * QT, C], BF16) for u in range(NS)]
    RG = [[0, 1, 2, 3], [4, 5, 6, 7]]

    with tile.TileContext(nc) as tc, ExitStack() as ctx:
        # ---------- pools: outer (whole kernel) ----------
        const = ctx.enter_context(tc.tile_pool(name="const", bufs=1))
        mid = ctx.enter_context(tc.tile_pool(name="mid", bufs=1))
        sm = ctx.enter_context(tc.tile_pool(name="sm", bufs=2))

        # ---------- constants ----------
        ident_bf = const.tile([P, P], BF16)
        make_identity(nc, ident_bf)
        eps_t = const.tile([P, 1], F32)
        nc.vector.memset(eps_t, EPS)
        ones64 = const.tile([P, D], F32)
        nc.vector.memset(ones64, 1.0)

        # per-feature bias tiles
        bq_sb = const.tile([P, 8], F32)     # (b_attn[0:C], scale pre-folded) -> [128, 8]
        nc.sync.dma_start(out=bq_sb, in_=b_attn[0:C].rearrange("(f p) -> p f", p=P))
        bk_sb = const.tile([P, 8], F32)
        nc.sync.dma_start(out=bk_sb, in_=b_attn[C:2 * C].rearrange("(f p) -> p f", p=P))
        bfc_sb = const.tile([P, 32], F32)
        nc.sync.dma_start(out=bfc_sb, in_=b_fc[:].rearrange("(f p) -> p f", p=P))

        # qbase + iotas for mask building
        qbase_sb = const.tile([1, NS], F32)
        nc.sync.dma_start(out=qbase_sb, in_=qbase_in[:, :])
        kidx_i = const.tile([P, 1], I32)
        nc.gpsimd.iota(kidx_i, pattern=[[0, 1]], base=0, channel_multiplier=1)
        kidx_f = const.tile([P, 1], F32)
        nc.vector.tensor_copy(out=kidx_f, in_=kidx_i)
        qio_i = const.tile([1, QT], I32)
        nc.gpsimd.iota(qio_i, pattern=[[1, QT]], base=0, channel_multiplier=0)
        qio_f = const.tile([1, QT], F32)
        nc.vector.tensor_copy(out=qio_f, in_=qio_i)
        # ind[u][k, 2r+h, q] = 1 iff (qbase_u + q) >= 128*(4u+r) + k
        # (only the diagonal key-group g == u of slot u ever needs a mask)
        ind = []
        for u in range(NS):
            qg = sm.tile([1, QT], F32, name="qg", tag="qg")
            nc.vector.tensor_scalar_add(out=qg, in0=qio_f,
                                        scalar1=qbase_sb[0:1, u:u + 1])
            qgb = sm.tile([P, QT], F32, name="qgb", tag="qgb")
            nc.gpsimd.partition_broadcast(qgb, qg)
            qk2 = sm.tile([P, QT], F32, name="qk2", tag="qk2")
            nc.vector.tensor_scalar_sub(out=qk2, in0=qgb, scalar1=kidx_f)
            ind_u = const.tile([P, 8, QT], BF16, name=f"ind{u}")
            for r in range(4):
                for h in range(2):
                    nc.vector.tensor_scalar(out=ind_u[:, 4 * h + r:4 * h + r + 1, :], in0=qk2,
                                            scalar1=float(P * (4 * u + r)), scalar2=None,
                                            op0=ALU.is_ge)
            ind.append(ind_u)

        # ---------- persistent mid tiles ----------
        xb_sb = mid.tile([P, NS, C], BF16)   # x + b_o (proj residual, bf16)
        qT = mid.tile([P, 8, TOK], BF16)
        yT = mid.tile([P, 8, TOK], BF16)
        x2 = mid.tile([P, NS, C], F32)
        xln2T = mid.tile([P, 8, TOK], BF16)
        wo_sb = mid.tile([P, 8, C], BF16)
        ln2 = mid.tile([P, NS, C], BF16)
        NPRE = 2  # MLP weight m-groups streamed during attention

        def layer_norm(src, dst, s):
            stats = sm.tile([P, 2, 6], F32, name="lnstats", tag="lnstats")
            nc.vector.bn_stats(out=stats[:, 0, :], in_=src[:, s, 0:512])
            nc.vector.bn_stats(out=stats[:, 1, :], in_=src[:, s, 512:1024])
            mv = sm.tile([P, 2], F32, name="lnmv", tag="lnmv")
            nc.vector.bn_aggr(out=mv, in_=stats)
            rstd = sm.tile([P, 1], F32, name="lnrstd", tag="lnrstd")
            nc.scalar.activation(out=rstd, in_=mv[:, 1:2], func=AF.Sqrt, bias=eps_t, scale=1.0)
            nc.vector.reciprocal(out=rstd, in_=rstd)
            nc.vector.tensor_scalar(out=dst[:, s, :], in0=src[:, s, :],
                                    scalar1=mv[:, 0:1], scalar2=rstd,
                                    op0=ALU.subtract, op1=ALU.mult)

        def transpose_slot(lnt, s, dstT, tp_ps):
            for f in range(8):
                pt = tp_ps.tile([P, P], BF16, name="tpt", tag="tpt",
                                padded_shape=[P, 2 * P])
                nc.tensor.transpose(pt[:, :], lnt[:, s, P * f:P * (f + 1)], ident_bf)
                nc.vector.tensor_copy(out=dstT[:, f, QT * s:QT * (s + 1)], in_=pt[:, :])

        # ================= LN1 + QKV (slot-ordered K/V, early gathers) =========
        with tc.tile_pool(name="qkvp", bufs=1) as qp, \
             tc.tile_pool(name="kps_ps", bufs=2, space="PSUM") as kps_ps, \
             tc.tile_pool(name="qkv_ps", bufs=3, space="PSUM") as qkv_ps, \
             tc.tile_pool(name="tp_ps", bufs=2, space="PSUM") as tp_ps:
            x_sb = qp.tile([P, NS, C], F32)
            ln1 = qp.tile([P, NS, C], BF16)
            xlnT = qp.tile([P, 8, TOK], BF16)
            wk = qp.tile([P, 8, C], BF16)
            wv = qp.tile([P, 8, C], BF16)
            wq = qp.tile([P, 8, C], BF16)
            bv_bc = qp.tile([P, C], F32)
            bo_bc = qp.tile([P, C], F32)
            # x on sync/scalar queues
            for u in (0, 1, 2, 3):
                eng = nc.sync if u % 2 == 0 else nc.scalar
                eng.dma_start(out=x_sb[:, u, :], in_=x_in[QT * u:QT * (u + 1), :])
            nc.scalar.dma_start(out=wk, in_=w_attn[:, C:2 * C].rearrange("(kc kp) n -> kp kc n", kp=P))
            nc.scalar.dma_start(out=wv, in_=w_attn[:, 2 * C:3 * C].rearrange("(kc kp) n -> kp kc n", kp=P))
            nc.sync.dma_start(out=wq, in_=w_attn[:, 0:C].rearrange("(kc kp) n -> kp kc n", kp=P))
            nc.sync.dma_start(out=bv_bc, in_=b_attn[2 * C:3 * C].rearrange("(a c) -> a c", a=1).to_broadcast((P, C)))
            nc.sync.dma_start(out=bo_bc, in_=b_o[:].rearrange("(a c) -> a c", a=1).to_broadcast((P, C)))
            nc.scalar.dma_start(out=wo_sb, in_=w_o[:, :].rearrange("(kc kp) n -> kp kc n", kp=P))

            for u in (0, 1, 2, 3):
                layer_norm(x_sb, ln1, u)
                transpose_slot(ln1, u, xlnT, tp_ps)
                nc.vector.tensor_tensor(out=xb_sb[:, u, :], in0=x_sb[:, u, :],
                                        in1=bo_bc, op=ALU.add)

            # K then V per slot, earliest keys (u=0) first; fire gathers asap
            for u in (0, 1, 2, 3):
                for f in range(8):
                    kps = kps_ps.tile([P, QT], F32, name="kps", tag="kps", padded_shape=[P, 512])
                    for k in range(8):
                        nc.tensor.matmul(kps[:, :], wk[:, k, P * f:P * (f + 1)],
                                         xlnT[:, k, QT * u:QT * (u + 1)],
                                         start=(k == 0), stop=(k == 7))
                    kt_sb = sm.tile([P, QT], BF16, name="kt_sb", tag="kt_sb", bufs=3)
                    nc.vector.tensor_scalar_add(out=kt_sb, in0=kps, scalar1=bk_sb[:, f:f + 1])
                    nc.sync.dma_start(out=kt_in[u][P * f:P * (f + 1), :], in_=kt_sb)
                for n in range(2):
                    vps = qkv_ps.tile([P, 512], F32, name="vps", tag="qkvps")
                    for k in range(8):
                        nc.tensor.matmul(vps[:, :], xlnT[:, k, QT * u:QT * (u + 1)],
                                         wv[:, k, 512 * n:512 * (n + 1)],
                                         start=(k == 0), stop=(k == 7))
                    v_sb = sm.tile([P, 512], BF16, name="v_sb", tag="v_sb", bufs=3)
                    nc.vector.tensor_tensor(out=v_sb, in0=vps,
                                            in1=bv_bc[:, 512 * n:512 * (n + 1)], op=ALU.add)
                    nc.sync.dma_start(out=v_in[u][:, 512 * n:512 * (n + 1)], in_=v_sb)
                if mock_cc:
                    # sim-only path: SWDGE copies stand in for the collectives
                    nc.gpsimd.dma_start(out=kt_all[u][0:C, :], in_=kt_in[u][:, :])
                    nc.gpsimd.dma_start(out=v_all[u][0:QT, :], in_=v_in[u][:, :])
                else:
                    nc.gpsimd.collective_compute("AllGather", ALU.bypass,
                                                 ins=[kt_in[u][:, :]], outs=[kt_all[u][:, :]],
                                                 replica_groups=RG)
                    nc.gpsimd.collective_compute("AllGather", ALU.bypass,
                                                 ins=[v_in[u][:, :]], outs=[v_all[u][:, :]],
                                                 replica_groups=RG)

            # Q (feature-tile order); scale+bias folded on host into wq/bq
            for f in range(8):
                qps = qkv_ps.tile([P, TOK], F32, name="qps", tag="qkvps")
                for k in range(8):
                    nc.tensor.matmul(qps[:, :], wq[:, k, P * f:P * (f + 1)], xlnT[:, k, :],
                                     start=(k == 0), stop=(k == 7))
                nc.vector.tensor_scalar_add(out=qT[:, f, :], in0=qps, scalar1=bq_sb[:, f:f + 1])

        # MLP weights streamed during attention + fc2 output bias (these span
        # attention and MLP, reusing the space qkvp just freed)
        mlw = ctx.enter_context(tc.tile_pool(name="mlw", bufs=1))
        wfc_pre = [mlw.tile([P, 8, 512], BF16, name=f"wfcp{g}") for g in range(NPRE)]
        w2_pre = [mlw.tile([P, 4, 512], BF16, name=f"w2p{g}") for g in range(NPRE)]
        b2_bc = mlw.tile([P, C], F32)
        nc.sync.dma_start(out=b2_bc, in_=b_fc2[:].rearrange("(a c) -> a c", a=1).to_broadcast((P, C)))

        # ================= attention (key-group prefix pipeline) ============
        with tc.tile_pool(name="attp", bufs=1) as ap, \
             tc.tile_pool(name="at_ps", bufs=1, space="PSUM") as at_ps:
            # K/V gather loads; gather u holds global key tiles 4u+r
            ktb, vb = [], []
            for u in range(NS):
                ktb.append(ap.tile([P, 8, 4, QT], BF16, name=f"ktb{u}", tag=f"ktb{u}"))
                vb.append(ap.tile([P, 4, H, D + 1], BF16, name=f"vb{u}", tag=f"vb{u}"))
            for u in (0, 1, 2, 3):
                for r in range(4):
                    nc.sync.dma_start(
                        out=ktb[u][:, :, r, :],
                        in_=kt_all[u][C * r:C * (r + 1), :].rearrange("(j p) c -> p j c", p=P))
                    nc.sync.dma_start(
                        out=vb[u][:, r, :, 0:D],
                        in_=v_all[u][P * r:P * (r + 1), :].rearrange("p (h d) -> p h d", h=H))
                nc.vector.tensor_copy(out=vb[u][:, :, :, D:D + 1],
                                      in_=ones64.rearrange("p (a b) -> p a b", a=4))

            for s in range(NS):
                qsl = slice(QT * s, QT * (s + 1))
                for j in range(8):
                    # merged ya: h=0 half opens the bank (aligned start);
                    # h=1 half writes start=False onto pending-zero bytes
                    ya = at_ps.tile([D + 1, 2, QT], F32, name="ya", tag="ya", bufs=2,
                                    padded_shape=[D + 1, 2, 2 * QT])
                    nkt = 4 * (s + 1)
                    for g in range(s + 1):
                        # st planes 2r+h share two banks; plane 0/4 opens each
                        # bank with an aligned start=True, the rest accumulate
                        # onto pending-zero at (legal) unaligned offsets
                        st = at_ps.tile([P, 8, QT], F32, name="st", tag="st", bufs=2)
                        for h in range(2):
                            for r in range(4):
                                # bank h: uniform tile_position, opened by the
                                # aligned r==0 matmul; r>0 accumulate onto
                                # pending-zero at (legal) unaligned offsets
                                nc.tensor.matmul(
                                    st[:, 4 * h + r, :],
                                    ktb[g][64 * h:64 * (h + 1), j, r, :],
                                    qT[64 * h:64 * (h + 1), j, qsl],
                                    start=(r == 0), stop=(r == 3),
                                    tile_position=(64 * h, 0))
                        et = sm.tile([P, 8, QT], BF16, name="et", tag="et", bufs=3)
                        nc.scalar.activation(out=et, in_=st, func=AF.Exp, scale=1.0)
                        if g == s:  # diagonal group: apply causal mask
                            nc.vector.tensor_tensor(out=et, in0=et, in1=ind[s], op=ALU.mult)
                        if bisect == "B4":
                            continue
                        for r in range(4):
                            kt0 = 4 * g + r
                            for h in range(2):
                                nc.tensor.matmul(ya[:, h, :], vb[g][:, r, 2 * j + h, :],
                                                 et[:, 4 * h + r, :],
                                                 start=(kt0 == 0 and h == 0),
                                                 stop=(kt0 == nkt - 1 and h == 1))
                    if bisect in ("B4", "C"):
                        nc.vector.memset(yT[:, j, qsl], 0.001)
                        continue
                    # softmax divide
                    for h in range(2):
                        rc = sm.tile([1, QT], F32, name=f"rc{h}", tag=f"rc{h}")
                        nc.vector.reciprocal(out=rc, in_=ya[D:D + 1, h, :])
                        rb = sm.tile([D, QT], F32, name=f"rb{h}", tag=f"rb{h}")
                        nc.gpsimd.partition_broadcast(rb, rc)
                        nc.vector.tensor_tensor(out=yT[64 * h:64 * (h + 1), j, qsl],
                                                in0=ya[0:D, h, :], in1=rb, op=ALU.mult)

                # proj(s) + LN2(s), overlapped into the slot pipeline
                u = s
                for n in range(2):
                    ps = at_ps.tile([P, 512], F32, name="prps", tag="prps", bufs=2)
                    for k in range(8):
                        nc.tensor.matmul(ps[:, :], yT[:, k, qsl],
                                         wo_sb[:, k, 512 * n:512 * (n + 1)],
                                         start=(k == 0), stop=(k == 7))
                    nc.vector.tensor_tensor(out=x2[:, u, 512 * n:512 * (n + 1)], in0=ps[:, :],
                                            in1=xb_sb[:, u, 512 * n:512 * (n + 1)], op=ALU.add)
                layer_norm(x2, ln2, u)
                nc.vector.tensor_tensor(out=x2[:, u, :], in0=x2[:, u, :], in1=b2_bc, op=ALU.add)
                if u < NPRE:
                    nc.sync.dma_start(out=wfc_pre[u], in_=w_fc[:, 512 * u:512 * (u + 1)]
                                      .rearrange("(kc kp) n -> kp kc n", kp=P))
                    nc.sync.dma_start(out=w2_pre[u], in_=w_fc2[P * 4 * u:P * 4 * (u + 1), 0:512]
                                      .rearrange("(mc mp) n -> mp mc n", mp=P))

        # LN2 transposes (PSUM freed by attention scope close)
        with tc.tile_pool(name="tp2_ps", bufs=4, space="PSUM") as tp2_ps:
            for u in range(NS):
                transpose_slot(ln2, u, xln2T, tp2_ps)

        # ================= MLP =================
        with tc.tile_pool(name="mlpp", bufs=1) as mp, \
             tc.tile_pool(name="wmlp", bufs=2) as wmp:
            h_sb = mp.tile([P, 32, 512], BF16)
            for half in range(2):
                with tc.tile_pool(name=f"mlp_ps{half}", bufs=1, space="PSUM") as mlp_ps:
                    ops = [mlp_ps.tile([P, 512], F32, name=f"ops{t}", tag=f"ops{t}", bufs=1)
                           for t in range(NS)]
                    for m in range(32):
                        mg = m // 4
                        if half == 0:
                            if m % 4 == 0:
                                if mg < NPRE:
                                    wfc = wfc_pre[mg]
                                else:
                                    wfc = wmp.tile([P, 8, 512], BF16, name="wfc", tag="wfc")
                                    nc.sync.dma_start(out=wfc,
                                                      in_=w_fc[:, 512 * mg:512 * (mg + 1)]
                                                      .rearrange("(kc kp) n -> kp kc n", kp=P))
                            mo = P * (m % 4)
                            fps = mlp_ps.tile([P, 512], F32, name="fps", tag="fps", bufs=3)
                            for k in range(8):
                                nc.tensor.matmul(fps[:, :], wfc[:, k, mo:mo + P], xln2T[:, k, :],
                                                 start=(k == 0), stop=(k == 7))
                            nc.scalar.activation(out=h_sb[:, m, :], in_=fps[:, :],
                                                 func=(AF.Relu if sim_gelu_relu else AF.Gelu),
                                                 bias=bfc_sb[:, m:m + 1], scale=1.0)
                        if m % 4 == 0:
                            if half == 0 and mg < NPRE:
                                w2 = w2_pre[mg]
                            else:
                                w2 = wmp.tile([P, 4, 512], BF16, name="w2", tag="w2", bufs=3)
                                nc.scalar.dma_start(out=w2, in_=w_fc2[P * m:P * (m + 4),
                                                                      512 * half:512 * (half + 1)]
                                                    .rearrange("(mc mp) n -> mp mc n", mp=P))
                        for t in range(NS):
                            nc.tensor.matmul(ops[t][:, :], h_sb[:, m, P * t:P * (t + 1)],
                                             w2[:, m % 4, :], start=(m == 0), stop=(m == 31))
                    out_eng = [nc.sync, nc.scalar, nc.sync, nc.scalar]
                    for t in range(NS):
                        nc.vector.tensor_tensor(out=x2[:, t, 512 * half:512 * (half + 1)],
                                                in0=ops[t][:, :],
                                                in1=x2[:, t, 512 * half:512 * (half + 1)], op=ALU.add)
                        if half == 1:
                            out_eng[t].dma_start(out=out_ext[QT * t:QT * (t + 1), :], in_=x2[:, t, :])

    nc.finalize()
    return nc


def _get_nc():
    if "nc" not in _CACHE:
        _CACHE["nc"] = _build()
    return _CACHE["nc"]


def _prep(**inputs):
    f = lambda a: np.asarray(a, dtype=np.float32)
    x = f(inputs["x"])
    ln1_g, ln1_b = f(inputs["ln1_g"]), f(inputs["ln1_b"])
    ln2_g, ln2_b = f(inputs["ln2_g"]), f(inputs["ln2_b"])
    W_attn, b_attn = f(inputs["W_attn"]), f(inputs["b_attn"])
    W_o, b_o = f(inputs["W_o"]), f(inputs["b_o"])
    W_fc, b_fc = f(inputs["W_fc"]), f(inputs["b_fc"])
    W_fc2, b_fc2 = f(inputs["W_fc2"]), f(inputs["b_fc2"])

    # fold LN affine params into the next matmul; fold 1/sqrt(D) into W_q/b_q
    # (SCALE is a power of two, so the bf16 cast stays exact)
    W_attn_e = (ln1_g[:, None] * W_attn).copy()
    b_attn_e = (b_attn + ln1_b @ W_attn).copy()
    W_attn_e[:, 0:C] *= SCALE
    b_attn_e[0:C] *= SCALE
    W_fc_e = ln2_g[:, None] * W_fc
    b_fc_e = b_fc + ln2_b @ W_fc

    in_maps = []
    for r in range(N_CORES):
        b, p = divmod(r, 4)
        tiles = [p + 4 * u for u in range(NS)]
        xs = np.concatenate([x[b, QT * g:QT * (g + 1)] for g in tiles], axis=0)
        in_maps.append({
            "x": np.ascontiguousarray(xs),
            "qbase": np.array([[QT * g for g in tiles]], dtype=np.float32),
            "w_attn": W_attn_e.astype(ml_dtypes.bfloat16), "b_attn": b_attn_e,
            "w_o": W_o.astype(ml_dtypes.bfloat16), "b_o": b_o,
            "w_fc": W_fc_e.astype(ml_dtypes.bfloat16), "b_fc": b_fc_e,
            "w_fc2": W_fc2.astype(ml_dtypes.bfloat16), "b_fc2": b_fc2,
        })

    def assemble(results):
        out = np.empty((B, T, C), dtype=np.float32)
        for r in range(N_CORES):
            b, p = divmod(r, 4)
            o = results[r]["out"]
            for u in range(NS):
                g = p + 4 * u
                out[b, QT * g:QT * (g + 1)] = o[QT * u:QT * (u + 1)]
        return out

    return in_maps, assemble


def kernel(**inputs):
    from concourse.bass_utils import run_bass_kernel_spmd

    in_maps, assemble = _prep(**inputs)
    res = run_bass_kernel_spmd(_get_nc(), in_maps, list(range(N_CORES)))
    return assemble(res.results)
